# revision 24
# baseline (speedup 1.0000x reference)
"""Trainium2 Bass kernel for nn_DGN4 (gnn_message_passing)  -- v2.

Reference semantics (B=4, T=2048, D=256, K_SIM=8, K_CON=4):
  xn    = x / max(||x||, 1e-12)                       (row L2-normalize)
  sim   = xn @ xn^T, causally masked (strictly past), masked = -1e9
  A_sim = top-8 per row (one-hot), zeroed outside past
  A_con = "bottom-4" of sim excluding A_sim -- because masked/future
          entries score +1e9 in the negated space, the reference's con
          picks land on future columns (then zeroed by the causal mask)
          for every row with T - t >= 4.  Only rows T-3..T-1 get
          1..3 real con-neighbors.
  msg_* = degree-normalized mean of selected x rows
  ctx   = alpha*msg_pos + (1-alpha)*msg_neg
  delta = gelu(mix*x + (1-mix)*ctx) * scale   (exact erf gelu; per-channel
          gain/bias fold is skipped when gain==1, bias==0 -- the common case)

Sharding: 8 cores = 4 batches x 2 row-shards, one uniform SPMD program;
per-core differences are data only (odd cores get adjacent 128-row blocks
swapped so the same static tile offsets address their rows).

v2 performance notes (driven by the CoreSim cost model):
  - x is shipped as bf16: halves the serialized input-DMA time and makes
    every PE op (sim matmuls, transposes, aggregation) run at 1 cycle/row
    instead of fp32's 4, and DVE elementwise ops eligible for 2x modes.
  - the causal keep-mask is injected into PSUM by the PE itself:
    matmul(ps_tail, eye_bf, penalty_pattern, start=False) adds the 0/-1e9
    pattern onto the sim accumulation, so the psum->w copies are plain.
  - engine rebalance: psum->w copies on Act, top-8 on DVE, adjacency
    build (threshold stt) on Pool/gpsimd, transposes+aggregation on PE.
  - stage-skewed emission: sim(tile i) | select(tile i-1) | agg(tile i-2)
    so no engine stalls on the select chain of its own tile.
  - selection thresholds stay fp32 (w tile) -- bf16 sim values would tie
    at the 8th-largest and break the degree normalization.
"""

import numpy as np

B, T, D = 4, 2048, 256
PB = 128                 # partition block
NBLK = T // PB           # 16 row/col blocks per batch
NTILE = 8                # program tiles per core
# width (in 128-blocks) and own-block index per program tile; widths pair to 18
WB = [2, 16, 4, 14, 6, 12, 8, 10]
OWN = [0, 15, 2, 13, 4, 11, 6, 9]
NEG = -1.0e9
NEGF = -60000.0
POS = 1.0e9

# tile processing order (pipeline): tile 0/2 need only the first x blocks
# (earliest PE start), tile 1 (full width + contrarian chain) early so its
# long select pipeline overlaps the rest.
TORD = [0, 2, 1, 3, 5, 7, 6, 4]

_PROGRAMS = {}


def _build_patterns():
    """Penalty patterns (0 = keep, NEG = masked) for the last two 128-col
    blocks of each program tile, as a function of tile parity (k%2) and
    core parity.  patterns[parity] is [2, PB, 2*PB] float32."""
    tri = np.where(np.arange(PB)[None, :] < np.arange(PB)[:, None], 0.0, NEGF)
    keep = np.zeros((PB, PB), np.float32)
    mask = np.full((PB, PB), NEGF, np.float32)
    out = []
    for parity in (0, 1):
        m = np.zeros((2, PB, 2 * PB), np.float32)
        if parity == 0:
            m[0] = np.concatenate([tri, mask], axis=1)   # even k
            m[1] = np.concatenate([keep, tri], axis=1)   # odd k
        else:
            m[0] = np.concatenate([tri, keep], axis=1)
            m[1] = np.concatenate([mask, tri], axis=1)
        out.append(m.astype(np.float32))
    return out


def _build_program(unit_affine=True):
    import concourse.bacc as bacc
    import concourse.tile as tile
    from concourse import mybir

    f32 = mybir.dt.float32
    f16 = mybir.dt.float16
    Alu = mybir.AluOpType
    Act = mybir.ActivationFunctionType

    nc = bacc.Bacc(None)
    x_ext = nc.declare_dram_parameter("x", [T, D], f16, isOutput=False)
    pat_ext = nc.declare_dram_parameter("patterns", [2, PB, 2 * PB], f16, isOutput=False)
    consts_ext = nc.declare_dram_parameter("consts", [PB, 8], f32, isOutput=False)
    eye_ext = nc.declare_dram_parameter("eye_bf", [PB, PB], f16, isOutput=False)
    if not unit_affine:
        gain_ext = nc.declare_dram_parameter("gain_bc", [PB, D], f32, isOutput=False)
        bias_ext = nc.declare_dram_parameter("bias_bc", [PB, D], f32, isOutput=False)
    out_ext = nc.declare_dram_parameter("out", [NTILE * PB, D], f32, isOutput=True)

    with tile.TileContext(nc) as tc:
        with (
            tc.tile_pool(name="singles", bufs=1) as singles,
            tc.tile_pool(name="scr", bufs=4) as scr,
            tc.tile_pool(name="wp", bufs=4) as wp,
            tc.tile_pool(name="t1p", bufs=3) as t1p,
            tc.tile_pool(name="ap", bufs=5) as apool,
            tc.tile_pool(name="atp", bufs=3) as atpool,
            tc.tile_pool(name="small", bufs=6) as small,
            tc.tile_pool(name="bl", bufs=4) as blp,
            tc.tile_pool(name="ps_sim", bufs=4, space="PSUM") as ps_sim,
            tc.tile_pool(name="ps_t", bufs=2, space="PSUM") as ps_t,
            tc.tile_pool(name="ps_ctx", bufs=2, space="PSUM") as ps_ctx,
        ):
            # ---- input DMAs (transfers serialize on the DMA engines; order
            # matters: first blocks + patterns first so compute starts early)
            x_all = singles.tile([PB, NBLK, D], f16)
            x_re = x_ext[:].rearrange("(c p) d -> p c d", p=PB)
            nc.sync.dma_start(out=x_all[:, 0:2, :], in_=x_re[:, 0:2, :])
            pat_sb = singles.tile([PB, 2, 2 * PB], f16)
            nc.sync.dma_start(out=pat_sb, in_=pat_ext[:].rearrange("q p m -> p q m"))
            eye_sb = singles.tile([PB, PB], f16)
            nc.sync.dma_start(out=eye_sb, in_=eye_ext[:])
            consts_sb = singles.tile([PB, 8], f32)
            nc.sync.dma_start(out=consts_sb, in_=consts_ext[:])
            for g in range(1, 8):
                nc.sync.dma_start(out=x_all[:, 2 * g:2 * g + 2, :],
                                  in_=x_re[:, 2 * g:2 * g + 2, :])
            if not unit_affine:
                gain_sb = singles.tile([PB, D], f32)
                nc.sync.dma_start(out=gain_sb, in_=gain_ext[:])
                bias_sb = singles.tile([PB, D], f32)
                nc.sync.dma_start(out=bias_sb, in_=bias_ext[:])

            # first-touch copies: TensorScalar-family instructions encode only
            # one sync wait, so no TS op may be the first on its engine to
            # observe two DMA queues.  TensorCopy tolerates multiple waits.
            touch_b = singles.tile([PB, 4], f16)
            touch_f = singles.tile([PB, 2], f32)
            nc.vector.tensor_copy(touch_b[:, 0:1], x_all[:, 0, 0:1])
            nc.vector.tensor_copy(touch_b[:, 1:2], pat_sb[:, 0, 0:1])
            nc.vector.tensor_copy(touch_b[:, 2:3], eye_sb[:, 0:1])
            nc.vector.tensor_copy(touch_f[:, 0:1], consts_sb[:, 0:1])
            touch_p = singles.tile([PB, 2], f16)
            touch_pf = singles.tile([PB, 1], f32)
            nc.gpsimd.tensor_copy(touch_p[:, 0:1], x_all[:, 0, 0:1])
            nc.gpsimd.tensor_copy(touch_p[:, 1:2], pat_sb[:, 0, 0:1])
            nc.gpsimd.tensor_copy(touch_pf, consts_sb[:, 0:1])

            mix_ap = consts_sb[:, 0:1]
            alpha1m_ap = consts_sb[:, 2:3]      # alpha*(1-mix)
            onemalpha1m_ap = consts_sb[:, 3:4]  # (1-alpha)*(1-mix)
            scale_ap = consts_sb[:, 4:5]

            # PE first-touch of eye (DMA queue) so real transposes stay
            # within the fused-matmul wait budget
            eye_touch = ps_t.tile([PB, PB], f16, tag="pst")
            nc.tensor.transpose(eye_touch, eye_sb, eye_sb)

            # pre-load Act function tables (Square, Sqrt, Gelu) while DMAs
            # stream so no table load blocks the Act pipeline mid-kernel
            tbl = singles.tile([PB, 3], f32)
            nc.scalar.activation(tbl[:, 0:1], consts_sb[:, 0:1], Act.Square)
            nc.scalar.activation(tbl[:, 1:2], consts_sb[:, 0:1], Act.Sqrt)
            nc.scalar.activation(tbl[:, 2:3], consts_sb[:, 0:1], Act.Gelu)

            # mix * eye (f16): lets the PE accumulate mix*x onto ctx in PSUM
            mixeye = singles.tile([PB, PB], f16)
            nc.vector.tensor_scalar_mul(mixeye, eye_sb, mix_ap)

            # ---- grouped prologue: norms + normalize + transpose ---------
            # xnT layout: [d-half (partition), block, half, token] so each
            # 8-transpose psT batch lands in one contiguous [PB,1024] copy.
            nrm2 = singles.tile([PB, NBLK], f32)
            rinv = singles.tile([PB, NBLK], f32)
            xn_all = singles.tile([PB, NBLK, D], f16)
            xnT = singles.tile([PB, NBLK, 2, PB], f16)
            for grp in range(4):
                cs = range(grp * 4, (grp + 1) * 4)
                for c in cs:
                    # norm^2: split across engines (DVE stt / Act Square)
                    if c % 2 == 0:
                        sq = scr.tile([PB, D], f16, tag="sq")
                        nc.vector.scalar_tensor_tensor(
                            sq, x_all[:, c, :], 1.0, x_all[:, c, :],
                            op0=Alu.mult, op1=Alu.mult,
                            accum_out=nrm2[:, c:c + 1])
                    else:
                        sqf = scr.tile([PB, D], f32, tag="sqf")
                        nc.scalar.activation(sqf, x_all[:, c, :], Act.Square,
                                             accum_out=nrm2[:, c:c + 1])
                g4 = slice(grp * 4, grp * 4 + 4)
                nc.vector.tensor_scalar_max(nrm2[:, g4], nrm2[:, g4], 1e-24)
                nrmg = scr.tile([PB, 4], f32, tag="nrmg")
                nc.scalar.activation(nrmg, nrm2[:, g4], Act.Sqrt)
                nc.vector.reciprocal(rinv[:, g4], nrmg)
                for c in cs:
                    if c % 2 == 0:
                        nc.vector.tensor_scalar_mul(
                            xn_all[:, c, :], x_all[:, c, :], rinv[:, c:c + 1])
                    else:
                        nc.scalar.activation(xn_all[:, c, :], x_all[:, c, :],
                                             Act.Copy, scale=rinv[:, c:c + 1])
                psT = ps_t.tile([PB, 1024], f16, tag="pst")
                for u, c in enumerate(cs):
                    for hf in (0, 1):
                        nc.tensor.transpose(
                            psT[:, (2 * u + hf) * PB:(2 * u + hf + 1) * PB],
                            xn_all[:, c, hf * PB:(hf + 1) * PB], eye_sb)
                dst = xnT[:, grp * 4:grp * 4 + 4, :, :]
                if grp % 2 == 0:
                    nc.vector.tensor_copy(dst, psT)
                else:
                    nc.scalar.copy(dst, psT)

            # ---- per-tile pipeline stages --------------------------------
            state = {}
            wcopy_flip = [0]

            def stage_sim(k):
                nb = WB[k]
                W = nb * PB
                own = OWN[k]
                w_t = wp.tile([PB, W], f32, tag="w")
                n512 = (W + 511) // 512
                for j in range(n512):
                    lo = j * 512
                    n = min(512, W - lo)
                    cb, ncb = lo // PB, n // PB
                    ps = ps_sim.tile([PB, n], f32, tag="ps_sim")
                    last_chunk = (j == n512 - 1)
                    nc.tensor.matmul(
                        ps, xnT[:, own, 0, :],
                        xnT[:, cb:cb + ncb, 0, :], start=True, stop=False)
                    if last_chunk:
                        # PE adds the causal penalty pattern onto the tail:
                        # eye^T @ pattern == pattern
                        nc.tensor.matmul(
                            ps[:, n - 256:n], eye_sb, pat_sb[:, k % 2, :],
                            start=False, stop=False, skip_group_check=True)
                    nc.tensor.matmul(
                        ps, xnT[:, own, 1, :],
                        xnT[:, cb:cb + ncb, 1, :], start=False, stop=True)
                    # psum -> w (GPSIMD cannot access PSUM; Act does these)
                    nc.scalar.copy(w_t[:, lo:lo + n], ps)
                state[k] = {"w": w_t}

            def stage_select(k):
                nb = WB[k]
                W = nb * PB
                st = state[k]
                w_t = st["w"]
                v8 = small.tile([PB, 8], f32, tag="v8")
                nc.vector.max(out=v8, in_=w_t)
                tau = small.tile([PB, 1], f32, tag="tau")
                nc.vector.tensor_scalar_max(tau, v8[:, 7:8], -1e4)
                cnt8 = small.tile([PB, 8], f32, tag="cnt8")
                deg = small.tile([PB, 1], f32, tag="deg")
                nc.vector.tensor_scalar(cnt8, v8, -1e4, None, op0=Alu.is_gt,
                                        op1=Alu.add, accum_out=deg)
                nc.vector.tensor_scalar_max(deg, deg, 1.0)
                coef = small.tile([PB, 1], f32, tag="coef")
                nc.vector.reciprocal(coef, deg)
                nc.vector.tensor_scalar(coef, coef, alpha1m_ap, None, op0=Alu.mult)

                # A_scaled = (w >= tau) * (alpha*(1-mix)/deg), f16
                A_t = apool.tile([PB, W], f16, tag="A")
                nc.gpsimd.memzero(A_t[:, 0:2])
                nc.gpsimd.tensor_scalar(A_t, w_t, tau, coef,
                                        op0=Alu.is_ge, op1=Alu.mult)
                st["A"] = A_t

                # contrarian branch: only the full-width tile can have any
                # (rows T-3..T-1); exact zeros elsewhere by construction.
                if k == 1:
                    wneg = t1p.tile([PB, W], f32, tag="t1")
                    nc.gpsimd.tensor_scalar_mul(wneg, w_t, -1.0)
                    negv8 = small.tile([PB, 8], f32, tag="negv8")
                    nc.vector.tensor_scalar_mul(negv8, v8, -1.0)
                    w2 = t1p.tile([PB, W], f32, tag="t1")
                    nc.vector.match_replace(out=w2, in_to_replace=negv8,
                                            in_values=wneg, imm_value=NEG)
                    vc8 = small.tile([PB, 8], f32, tag="vc8")
                    nc.vector.max(out=vc8, in_=w2)
                    cnt4 = small.tile([PB, 4], f32, tag="cnt4")
                    degc = small.tile([PB, 1], f32, tag="degc")
                    nc.vector.tensor_scalar(cnt4, vc8[:, 0:4], 1e4, None,
                                            op0=Alu.is_lt, op1=Alu.add,
                                            accum_out=degc)
                    nc.vector.tensor_scalar_max(degc, degc, 1.0)
                    coefc = small.tile([PB, 1], f32, tag="coefc")
                    nc.vector.reciprocal(coefc, degc)
                    nc.vector.tensor_scalar(coefc, coefc, onemalpha1m_ap, None,
                                            op0=Alu.mult)
                    # hi = (w2 < 1e8) * coefc   (zero for future sentinels)
                    hi_t = t1p.tile([PB, W], f32, tag="t1")
                    nc.gpsimd.tensor_scalar(hi_t, w2, 1e4, coefc,
                                            op0=Alu.is_lt, op1=Alu.mult)
                    Ac_t = apool.tile([PB, W], f16, tag="A")
                    nc.vector.scalar_tensor_tensor(
                        Ac_t, w2, vc8[:, 3:4], hi_t,
                        op0=Alu.is_ge, op1=Alu.mult)
                    st["Ac"] = Ac_t

            def transpose_to(src, dst, nblocks, copy_eng_iter):
                for g in range(0, nblocks, 8):
                    cnt = min(8, nblocks - g)
                    psT = ps_t.tile([PB, 1024], f16, tag="pst")
                    for u in range(cnt):
                        nc.tensor.transpose(
                            psT[:, u * PB:(u + 1) * PB],
                            src[:, (g + u) * PB:(g + u + 1) * PB], eye_sb)
                    eng = next(copy_eng_iter)
                    if eng == "v":
                        nc.vector.tensor_copy(dst[:, g * PB:(g + cnt) * PB],
                                              psT[:, 0:cnt * PB])
                    else:
                        nc.scalar.copy(dst[:, g * PB:(g + cnt) * PB],
                                       psT[:, 0:cnt * PB])

            def _alternator():
                # DVE gets 2 of 3 bf16 copies (2x perf mode), Act the third
                while True:
                    yield "v"
                    yield "v"
                    yield "s"

            copy_eng = _alternator()

            def stage_agg(k):
                nb = WB[k]
                W = nb * PB
                own = OWN[k]
                st = state[k]
                A_t = st["A"]
                Ac_t = st.get("Ac")

                AT_t = atpool.tile([PB, W], f16, tag="AT")
                transpose_to(A_t, AT_t, nb, copy_eng)
                if Ac_t is not None:
                    ATc_t = atpool.tile([PB, W], f16, tag="AT")
                    transpose_to(Ac_t, ATc_t, nb, copy_eng)

                # ctx accumulation; the mixeye matmul folds the mix*x blend
                # term into the same PSUM group (ctx coefs carry (1-mix)).
                ctx_ps = ps_ctx.tile([PB, D], f32, tag="ctx")
                nc.tensor.matmul(ctx_ps, mixeye, x_all[:, own, :],
                                 start=True, stop=False)
                for c in range(nb):
                    nc.tensor.matmul(ctx_ps, AT_t[:, c * PB:(c + 1) * PB],
                                     x_all[:, c, :], start=False,
                                     stop=(Ac_t is None and c == nb - 1))
                if Ac_t is not None:
                    for c in range(nb):
                        nc.tensor.matmul(ctx_ps, ATc_t[:, c * PB:(c + 1) * PB],
                                         x_all[:, c, :],
                                         start=False, stop=(c == nb - 1))

                # gelu straight off PSUM (Act), then scale on Pool, then DMA
                if unit_affine:
                    g_t = blp.tile([PB, D], f32, tag="g")
                    nc.scalar.activation(g_t, ctx_ps, Act.Gelu)
                else:
                    z_t = blp.tile([PB, D], f32, tag="z")
                    nc.vector.tensor_mul(z_t, ctx_ps, gain_sb)
                    nc.vector.tensor_add(z_t, z_t, bias_sb)
                    g_t = blp.tile([PB, D], f32, tag="g")
                    nc.scalar.activation(g_t, z_t, Act.Gelu)
                d_t = blp.tile([PB, D], f32, tag="d")
                nc.gpsimd.tensor_scalar(d_t, g_t, scale_ap, None, op0=Alu.mult)
                nc.sync.dma_start(out=out_ext[k * PB:(k + 1) * PB, :], in_=d_t)
                del state[k]

            # stage-skewed emission: sim(i) | select(i-1) | agg(i-3)
            for i in range(len(TORD) + 3):
                if i < len(TORD):
                    stage_sim(TORD[i])
                if 1 <= i <= len(TORD):
                    stage_select(TORD[i - 1])
                if i >= 3:
                    stage_agg(TORD[i - 3])

    nc.compile()
    return nc


def _get_program(unit_affine=True):
    key = bool(unit_affine)
    if key not in _PROGRAMS:
        _PROGRAMS[key] = _build_program(unit_affine=key)
    return _PROGRAMS[key]


def _make_in_maps(inputs):
    """Host-side prep: returns (in_maps for cores 0-7, unit_affine flag)."""
    x = np.asarray(inputs["x"], dtype=np.float32)
    gain = np.asarray(inputs["gain"], dtype=np.float32).reshape(D)
    bias = np.asarray(inputs["bias"], dtype=np.float32).reshape(D)
    log_mix = float(np.asarray(inputs["log_mix"]))
    log_alpha = float(np.asarray(inputs["log_alpha"]))
    log_scale = float(np.asarray(inputs["log_scale"]))

    mix = np.float32(1.0 / (1.0 + np.exp(-np.float64(log_mix))))
    alpha = np.float32(1.0 / (1.0 + np.exp(-np.float64(log_alpha))))
    scale = np.float32(np.logaddexp(0.0, np.float64(log_scale)) + 0.01)
    unit_affine = bool(np.all(gain == 1.0) and np.all(bias == 0.0))

    consts = np.zeros((PB, 8), np.float32)
    consts[:, 0] = mix
    consts[:, 1] = np.float32(1.0) - mix
    consts[:, 2] = alpha * (np.float32(1.0) - mix)
    consts[:, 3] = (np.float32(1.0) - alpha) * (np.float32(1.0) - mix)
    consts[:, 4] = scale
    eye_bf = np.eye(PB, dtype=np.float32).astype(np.float16)
    patterns = _build_patterns()

    swap_perm = np.arange(NBLK).reshape(-1, 2)[:, ::-1].reshape(-1)

    in_maps = []
    for c in range(8):
        b, p = c // 2, c % 2
        xb = x[b]
        if p:
            xb = xb.reshape(NBLK, PB, D)[swap_perm].reshape(T, D)
        im = {
            "x": np.ascontiguousarray(xb.astype(np.float16)),
            "patterns": np.ascontiguousarray(
                patterns[p].astype(np.float16)),
            "consts": consts,
            "eye_bf": eye_bf,
        }
        if not unit_affine:
            im["gain_bc"] = np.ascontiguousarray(
                np.broadcast_to(gain[None, :], (PB, D)).astype(np.float32))
            im["bias_bc"] = np.ascontiguousarray(
                np.broadcast_to(bias[None, :], (PB, D)).astype(np.float32))
        in_maps.append(im)
    return in_maps, unit_affine


def kernel(**inputs):
    in_maps, unit_affine = _make_in_maps(inputs)
    from concourse.bass_utils import run_bass_kernel_spmd
    nc = _get_program(unit_affine)
    res = run_bass_kernel_spmd(nc, in_maps, list(range(8))).results

    out = np.empty((B, T, D), np.float32)
    for c in range(8):
        b, p = c // 2, c % 2
        o = np.asarray(res[c]["out"])
        for k in range(NTILE):
            g_act = OWN[k] ^ p
            out[b, g_act * PB:(g_act + 1) * PB, :] = o[k * PB:(k + 1) * PB, :]
    return out
